# revision 26
# baseline (speedup 1.0000x reference)
"""Causal self-attention (B=2, T=2048, C=2048, H=16, D=128) on 8 TRN2 NeuronCores.

Sharding: 8 cores = 2 batches x 4 head-groups (4 heads each).
Core m: batch b = m // 4, heads [4g, 4g+4) with g = m % 4.
  - c_attn columns split by head (tensor parallel), c_proj rows split by head.
  - Each core returns a partial projection output; host sums the 4 partials
    per batch and adds b_proj (the unshard step for row-parallel c_proj).

Per-core pipeline (all matmuls in float32r: full PE speed, ~1e-4 accuracy):
  A1: transpose x [T, C] -> xT [C, T] via PE matmuls against identity
  A2: QT/KT/VT = (x @ W)^T (transposed orientation, [d, tok]) -> DRAM scratch
  B:  per head: transpose VT -> V [k, d]; ST = K Q^T chunk pairs -> exp ->
      (mask-mul on diagonal pairs) -> yT += V_chunk^T @ PT, sums += ones^T @ PT;
      1/sum = exp(-ln(sum)) on ACT; emission software-pipelined (chunk-pair lag)
      so the PE never queues behind ACT.
  C:  out = concat_heads(y) @ Wp_rows (partial) -> DRAM output
"""
import sys

sys.path.insert(0, "/opt/trn_rl_repo")
sys.path.insert(0, "/root/.axon_site")

import numpy as np

N_EMBD = 2048
N_HEAD = 16
HEAD_DIM = 128
B, T = 2, 2048
N_CORES = 8
H_PER_CORE = 4          # heads per core
HD = H_PER_CORE * HEAD_DIM  # 512: per-core q/k/v width
NC_C = N_EMBD // 128    # 16 contraction chunks
NT = T // 128           # 16 token 128-blocks
NQB = T // 512          # 4 q blocks of 512
SCALE = 1.0 / np.sqrt(HEAD_DIM)

_CACHE = {}


def _build():
    import concourse.bacc as bacc
    import concourse.mybir as mybir
    import concourse.tile as tile

    f32 = mybir.dt.float32
    f32r = mybir.dt.float32r
    Exp = mybir.ActivationFunctionType.Exp
    Ln = mybir.ActivationFunctionType.Ln
    Ident = mybir.ActivationFunctionType.Identity

    nc = bacc.Bacc("TRN2", target_bir_lowering=False, debug=False, num_devices=N_CORES)

    x_dram = nc.dram_tensor("x", [T, N_EMBD], f32, kind="ExternalInput").ap()
    wq_dram = nc.dram_tensor("wq", [N_EMBD, HD], f32, kind="ExternalInput").ap()
    wk_dram = nc.dram_tensor("wk", [N_EMBD, HD], f32, kind="ExternalInput").ap()
    wv_dram = nc.dram_tensor("wv", [N_EMBD, HD], f32, kind="ExternalInput").ap()
    bq_dram = nc.dram_tensor("bq", [HD, 1], f32, kind="ExternalInput").ap()
    bk_dram = nc.dram_tensor("bk", [HD, 1], f32, kind="ExternalInput").ap()
    bv_dram = nc.dram_tensor("bv", [HD, 1], f32, kind="ExternalInput").ap()
    wp_dram = nc.dram_tensor("wp", [HD, N_EMBD], f32, kind="ExternalInput").ap()
    ident_dram = nc.dram_tensor("ident", [128, 256], f32, kind="ExternalInput").ap()
    ones_dram = nc.dram_tensor("ones", [128, 1], f32, kind="ExternalInput").ap()
    onesr_dram = nc.dram_tensor("onesr", [1, 128], f32, kind="ExternalInput").ap()
    mmask_dram = nc.dram_tensor("mmask", [128, 2, 1024], f32, kind="ExternalInput").ap()
    out_dram = nc.dram_tensor("out", [T, N_EMBD], f32, kind="ExternalOutput").ap()

    with tile.TileContext(nc) as tc:
        with tc.tile_pool(name="singles", bufs=1) as singles, \
             tc.tile_pool(name="hin", bufs=2) as hin_pool, \
             tc.tile_pool(name="dram", bufs=1, space="DRAM") as dram:

            ident = singles.tile([128, 256], f32r)  # [I | 0]
            nc.sync.dma_start(ident[:], ident_dram[:].bitcast(f32r))

            qt_scr = dram.tile([HD, T], f32r, name="qt_scr")
            kt_scr = dram.tile([HD, T], f32r, name="kt_scr")
            vt_scr = dram.tile([HD, T], f32r, name="vt_scr")

            def load_head(h):
                # hin pool is allocated before xt, so these DMAs only wait on
                # the A2 evac DMAs for head h's rows (no space-release dep).
                kt_h = hin_pool.tile([128, T], f32r, tag="kt_h", name=f"kt_h{h}")
                nc.sync.dma_start(kt_h[:], kt_scr[h * 128:(h + 1) * 128, :])
                qt_h = hin_pool.tile([128, T], f32r, tag="qt_h", name=f"qt_h{h}")
                nc.sync.dma_start(qt_h[:], qt_scr[h * 128:(h + 1) * 128, :])
                vt_h = hin_pool.tile([128, T], f32r, tag="vt_h", name=f"vt_h{h}", bufs=1)
                nc.sync.dma_start(vt_h[:], vt_scr[h * 128:(h + 1) * 128, :])
                return kt_h, qt_h, vt_h

            # head 0's q/k/v tiles are SBUF-resident targets for A2's evacs
            kt_h0 = hin_pool.tile([128, T], f32r, tag="kt_h", name="kt_h0")
            qt_h0 = hin_pool.tile([128, T], f32r, tag="qt_h", name="qt_h0")
            vt_h0 = hin_pool.tile([128, T], f32r, tag="vt_h", name="vt_h0", bufs=1)
            h0_tiles = {0: qt_h0, 1: kt_h0, 2: vt_h0}

            # ---------------- Phase A ----------------
            with tc.tile_pool(name="xt", bufs=1) as xt_pool:
                xt = []  # 16 tiles [128 c, T]
                for c in range(NC_C):
                    t = xt_pool.tile([128, T], f32r, tag=f"xt{c}", name=f"xt{c}")
                    xt.append(t)

                # A1: transpose x into xT, row-tile pairs. x blocks stationary,
                # identity moving: out[c, t'] = sum_t x[t, c] * I[t, t']
                with tc.tile_pool(name="xin", bufs=4) as xin_pool, \
                     tc.tile_pool(name="psA1", bufs=3, space="PSUM") as psA1:
                    for tq in range(NT // 2):
                        xrow = []
                        for dt_ in range(2):
                            xr = xin_pool.tile([128, N_EMBD], f32r, tag="xin", name=f"xin{tq}_{dt_}")
                            tglob = tq * 2 + dt_
                            nc.sync.dma_start(xr[:], x_dram[tglob * 128:(tglob + 1) * 128, :].bitcast(f32r))
                            xrow.append(xr)
                        for c in range(NC_C):
                            tp = psA1.tile([128, 512], f32, tag="tp", name=f"tp{tq}_{c}")
                            for dt_ in range(2):
                                nc.tensor.matmul(
                                    tp[:, dt_ * 256:(dt_ + 1) * 256],
                                    xrow[dt_][:, c * 128:(c + 1) * 128],
                                    ident[:],
                                    start=True, stop=True,
                                )
                            src = tp.rearrange("p (a b) -> p a b", a=2)[:, :, 0:128]
                            dst = xt[c][:, tq * 256:(tq + 1) * 256].rearrange(
                                "p (a b) -> p a b", a=2)
                            if c % 2 == 0:
                                nc.scalar.copy(dst, src)
                            else:
                                nc.vector.tensor_copy(dst, src)

                # biases (loaded late so they don't delay A1's x DMAs)
                bias_t = singles.tile([128, 3 * H_PER_CORE], f32)
                nc.sync.dma_start(bias_t[:, 0:4], bq_dram.rearrange("(a p) o -> p (a o)", p=128))
                nc.sync.dma_start(bias_t[:, 4:8], bk_dram.rearrange("(a p) o -> p (a o)", p=128))
                nc.sync.dma_start(bias_t[:, 8:12], bv_dram.rearrange("(a p) o -> p (a o)", p=128))
                ones_col = singles.tile([128, 1], f32r)
                nc.sync.dma_start(ones_col[:], ones_dram[:].bitcast(f32r))
                ones_row = singles.tile([1, 128], f32r)
                nc.sync.dma_start(ones_row[:], onesr_dram[:].bitcast(f32r))

                # A2: QT/KT/VT (transposed orientation) -> DRAM scratch
                with tc.tile_pool(name="wqkv", bufs=16) as wqkv_pool, \
                     tc.tile_pool(name="psA2", bufs=2, space="PSUM") as psA2, \
                     tc.tile_pool(name="evA2", bufs=6) as evA2:
                    od_order = [(kind, hh) for hh in range(H_PER_CORE) for kind in range(3)]
                    for kind, od_l in od_order:
                        od = kind * H_PER_CORE + od_l  # bias column index
                        w_dram_src = (wq_dram, wk_dram, wv_dram)[kind]
                        dst = (qt_scr, kt_scr, vt_scr)[kind]
                        psums = []
                        for tqb in range(NQB):
                            p = psA2.tile([128, 512], f32, tag=f"qk{tqb}", name=f"qk{od}_{tqb}")
                            psums.append(p)
                        for c in range(NC_C):
                            w = wqkv_pool.tile([128, 128], f32r, tag="wqkv", name=f"wqkv{od}_{c}")
                            nc.sync.dma_start(
                                w[:], w_dram_src[c * 128:(c + 1) * 128,
                                                 od_l * 128:(od_l + 1) * 128].bitcast(f32r))
                            for tqb in range(NQB):
                                nc.tensor.matmul(
                                    psums[tqb][:], w[:], xt[c][:, tqb * 512:(tqb + 1) * 512],
                                    start=(c == 0), stop=(c == NC_C - 1),
                                )
                        for tqb in range(NQB):
                            if od_l == 0:
                                # head 0: evacuate straight into the resident tile
                                nc.scalar.activation(
                                    h0_tiles[kind][:, tqb * 512:(tqb + 1) * 512],
                                    psums[tqb][:], bias=bias_t[:, od:od + 1],
                                    func=Ident, scale=1.0)
                            else:
                                ev = evA2.tile([128, 512], f32r, tag="evqk", name=f"evA2_{od}_{tqb}")
                                nc.scalar.activation(
                                    ev[:], psums[tqb][:], Ident,
                                    bias=bias_t[:, od:od + 1], scale=1.0)
                                nc.sync.dma_start(
                                    dst[od_l * 128:(od_l + 1) * 128, tqb * 512:(tqb + 1) * 512], ev[:])


            # ---------------- Phases B & C ----------------
            with tc.tile_pool(name="ytc", bufs=1) as ytc_pool, \
                 tc.tile_pool(name="wp", bufs=1) as wp_pool, \
                 tc.tile_pool(name="bconst", bufs=1) as bconst:
                ytc = []  # resident normalized y^T tiles [128 d, 512 q] per (h, qb)
                for i in range(H_PER_CORE * NQB):
                    t = ytc_pool.tile([128, 512], f32r, tag=f"ytc{i}", name=f"ytc{i}")
                    ytc.append(t)
                wp_t = []
                mmask = bconst.tile([128, 2, 1024], f32r)

                with tc.tile_pool(name="vh", bufs=2) as vh_pool, \
                     tc.tile_pool(name="pt", bufs=8) as pt_pool, \
                     tc.tile_pool(name="ptm", bufs=4) as ptm_pool, \
                     tc.tile_pool(name="small", bufs=2) as small_pool, \
                     tc.tile_pool(name="psB", bufs=2, space="PSUM") as psB, \
                     tc.tile_pool(name="psB1", bufs=1, space="PSUM") as psB1:
                    deferred = []  # emission closures, flushed with a lag
                    rinv_box = {}

                    def flush(keep):
                        while len(deferred) > keep:
                            deferred.pop(0)()

                    def transpose_v(h, vt_h):
                        # VT [d, k] -> V chunks [128 k, 128 d] packed in [128, 16, 128]
                        # via wide-identity matmul (N=256 crosses the f32r speed cliff)
                        v_h = vh_pool.tile([128, NT, 128], f32r, tag="v_h", name=f"v_h{h}")
                        for g in range(8):
                            tvp = psB1.tile([128, 512], f32, tag="rbc", name=f"tvp{h}_{g}")
                            for kb in (2 * g, 2 * g + 1):
                                with nc.allow_low_precision(reason="transpose matmul f32r"):
                                    nc.tensor.matmul(
                                        tvp[:, (kb % 2) * 256:(kb % 2 + 1) * 256],
                                        vt_h[:, kb * 128:(kb + 1) * 128], ident[:],
                                        start=True, stop=True)
                            with nc.allow_low_precision(reason="v evac f32r"):
                                nc.vector.tensor_copy(
                                    v_h[:, 2 * g:2 * g + 2, :],
                                    tvp.rearrange("p (a b) -> p a b", a=2)[:, :, 0:128])
                        return v_h

                    head_tiles = {0: (kt_h0, qt_h0, vt_h0)}
                    nc.sync.dma_start(mmask[:], mmask_dram[:].bitcast(f32r))
                    for h in range(H_PER_CORE):
                        w = wp_pool.tile([128, N_EMBD], f32r, tag=f"wp{h}", name=f"wp{h}")
                        nc.sync.dma_start(w[:], wp_dram[h * 128:(h + 1) * 128, :].bitcast(f32r))
                        wp_t.append(w)

                    v_box = {}
                    for h in range(H_PER_CORE):
                        kt_h, qt_h, vt_h = head_tiles.pop(h)
                        if h == 0:
                            v_box[0] = transpose_v(0, vt_h)
                        v_all = v_box.pop(h)
                        if h + 1 < H_PER_CORE and h + 1 not in head_tiles:
                            head_tiles[h + 1] = load_head(h + 1)

                        for qb in reversed(range(NQB)):
                            i = h * NQB + qb
                            nkc = 4 * (qb + 1)
                            # at a stable point mid-head, emit next head's V transposes
                            if qb == 2 and h + 1 < H_PER_CORE:
                                def mk_tv(h2=h + 1, vt2=head_tiles[h + 1][2]):
                                    def tv():
                                        v_box[h2] = transpose_v(h2, vt2)
                                    return tv
                                deferred.append(mk_tv())
                            flush(keep=1)  # drain to 1 so prev qb's ln beats our exp to ACT
                            yt_ps = psB.tile([128, 512], f32, tag="yt", name=f"yt{h}_{qb}", bufs=2)
                            sum_ps = psB1.tile([1, 512], f32, tag="sum", name=f"sum{h}_{qb}")
                            for kp in range(nkc // 2):
                                st = psB.tile([128, 1024], f32, tag="st", name=f"st{h}_{qb}_{kp}")
                                for j in (0, 1):
                                    kc = 2 * kp + j
                                    nc.tensor.matmul(
                                        st[:, j * 512:(j + 1) * 512],
                                        kt_h[:, kc * 128:(kc + 1) * 128],
                                        qt_h[:, qb * 512:(qb + 1) * 512],
                                        start=True, stop=True,
                                    )
                                pt = pt_pool.tile([128, 1024], f32r, tag="pt",
                                                  name=f"pt{h}_{qb}_{kp}")
                                nc.scalar.activation(pt[:], st[:], Exp, scale=SCALE)
                                if kp >= 2 * qb:  # diagonal pair: multiplicative causal mask
                                    ptm = ptm_pool.tile([128, 1024], f32r, tag="ptm",
                                                        name=f"ptm{h}_{qb}_{kp}")
                                    with nc.allow_low_precision(reason="causal mask mul f32r"):
                                        nc.vector.tensor_mul(ptm[:], pt[:], mmask[:, kp - 2 * qb])
                                    src = ptm
                                else:
                                    src = pt

                                def consume(src=src, yt_ps=yt_ps, sum_ps=sum_ps, kp=kp,
                                            nkc=nkc, hh=h, h_=h, qb_=qb, v_ref=v_all,
                                            last=(kp == nkc // 2 - 1)):
                                    for j2 in (0, 1):
                                        kc2 = 2 * kp + j2
                                        nc.tensor.matmul(
                                            yt_ps[:], v_ref[:, kc2, :],
                                            src[:, j2 * 512:(j2 + 1) * 512],
                                            start=(kc2 == 0), stop=(kc2 == nkc - 1),
                                        )
                                        nc.tensor.matmul(
                                            sum_ps[:], ones_col[:],
                                            src[:, j2 * 512:(j2 + 1) * 512],
                                            start=(kc2 == 0), stop=(kc2 == nkc - 1),
                                        )
                                    if last:
                                        lnsum = small_pool.tile([1, 512], f32, tag="lnsum",
                                                                name=f"ln{h_}_{qb_}")
                                        nc.scalar.activation(lnsum[:], sum_ps[:], Ln)
                                        rinv = small_pool.tile([1, 512], f32r, tag="rinv",
                                                              name=f"ri{h_}_{qb_}")
                                        with nc.allow_low_precision(reason="exp(-ln) f32r"):
                                            nc.scalar.activation(rinv[:], lnsum[:], Exp,
                                                                 scale=-1.0)
                                        rinv_box[(h_, qb_)] = rinv

                                deferred.append(consume)
                                flush(keep=2)

                            def norm(i=i, yt_ps=yt_ps, h_=h, qb_=qb):
                                rinv = rinv_box.pop((h_, qb_))
                                rbc = psB1.tile([128, 512], f32, tag="rbc",
                                                name=f"rbc{h_}_{qb_}")
                                nc.tensor.matmul(rbc[:], ones_row[:], rinv[:],
                                                 start=True, stop=True)
                                rbc_sb = small_pool.tile([128, 512], f32r, tag="rbc_sb",
                                                         name=f"rbs{h_}_{qb_}")
                                with nc.allow_low_precision(reason="rbc copy f32r"):
                                    nc.vector.tensor_copy(rbc_sb[:], rbc[:])
                                with nc.allow_low_precision(reason="softmax normalize f32r"):
                                    nc.vector.tensor_mul(ytc[i][:], yt_ps[:], rbc_sb[:])

                            deferred.append(norm)
                            flush(keep=2)
                    flush(keep=0)

                # ---------------- Phase C ----------------
                with tc.tile_pool(name="oev", bufs=2) as oev_pool, \
                     tc.tile_pool(name="psC", bufs=2, space="PSUM") as psC:
                    for tb in range(NT):
                        qb, ts = tb // 4, (tb % 4) * 128
                        oev = oev_pool.tile([128, N_EMBD], f32, tag="oev", name=f"oev{tb}")
                        for ob in range(4):
                            po = psC.tile([128, 512], f32, tag=f"po{ob % 2}", name=f"po{tb}_{ob}")
                            for h in range(H_PER_CORE):
                                nc.tensor.matmul(
                                    po[:], ytc[h * NQB + qb][:, ts:ts + 128],
                                    wp_t[h][:, ob * 512:(ob + 1) * 512],
                                    start=(h == 0), stop=(h == H_PER_CORE - 1),
                                )
                            if ob % 2 == 0:
                                nc.scalar.copy(oev[:, ob * 512:(ob + 1) * 512], po[:])
                            else:
                                nc.vector.tensor_copy(oev[:, ob * 512:(ob + 1) * 512], po[:])
                        nc.sync.dma_start(out_dram[tb * 128:(tb + 1) * 128, :], oev[:])

    nc.compile()
    return nc


def _consts():
    mmask = np.zeros((128, 2, 2, 512), dtype=np.float32)
    for p in range(2):
        for j in range(2):
            kk = 128 * (2 * p + j) + np.arange(128)[:, None]
            qq = np.arange(512)[None, :]
            mmask[:, p, j, :] = np.where(qq >= kk, 1.0, 0.0)
    return {
        "ident": np.concatenate([np.eye(128, dtype=np.float32),
                  np.zeros((128, 128), np.float32)], axis=1),
        "ones": np.ones((128, 1), np.float32),
        "onesr": np.ones((1, 128), np.float32),
        "mmask": mmask.reshape(128, 2, 1024),
    }


def _run(inputs, trace=False):
    from concourse.bass_utils import run_bass_kernel_spmd

    if "nc" not in _CACHE:
        _CACHE["nc"] = _build()
    nc = _CACHE["nc"]

    x = np.asarray(inputs["x"], dtype=np.float32)
    W_attn = np.asarray(inputs["W_attn"], dtype=np.float32)
    b_attn = np.asarray(inputs["b_attn"], dtype=np.float32)
    W_proj = np.asarray(inputs["W_proj"], dtype=np.float32)
    b_proj = np.asarray(inputs["b_proj"], dtype=np.float32)

    consts = _consts()
    in_maps = []
    for m in range(N_CORES):
        b, g = m // 4, m % 4
        cs = g * HD
        im = {
            "x": np.ascontiguousarray(x[b]),
            "wq": np.ascontiguousarray(W_attn[:, cs:cs + HD]),
            "wk": np.ascontiguousarray(W_attn[:, N_EMBD + cs:N_EMBD + cs + HD]),
            "wv": np.ascontiguousarray(W_attn[:, 2 * N_EMBD + cs:2 * N_EMBD + cs + HD]),
            "bq": np.ascontiguousarray(b_attn[cs:cs + HD].reshape(HD, 1)),
            "bk": np.ascontiguousarray(b_attn[N_EMBD + cs:N_EMBD + cs + HD].reshape(HD, 1)),
            "bv": np.ascontiguousarray(b_attn[2 * N_EMBD + cs:2 * N_EMBD + cs + HD].reshape(HD, 1)),
            "wp": np.ascontiguousarray(W_proj[cs:cs + HD, :]),
        }
        im.update(consts)
        in_maps.append(im)

    res = run_bass_kernel_spmd(nc, in_maps, list(range(N_CORES)), trace=trace)
    out = np.zeros((B, T, N_EMBD), dtype=np.float32)
    for m in range(N_CORES):
        out[m // 4] += res.results[m]["out"]
    out += b_proj
    return out, res


def kernel(**inputs) -> np.ndarray:
    out, _ = _run(inputs, trace=False)
    return out
